# revision 21
# baseline (speedup 1.0000x reference)
"""Trainium2 Bass kernel for the Bahdanau-style band recurrence.

Math (per batch row b, position j, T=8 steps):
    g[j]   = W1 @ x[:, j] + b1 + b2                      (d=256)
    up[j]  <- relu(g[j] + W2 @ up[j-1])   (up[-1] = 0)
    dn[j]  <- relu(g[j] + W2 @ dn[j+1])   (dn[L]  = 0)
    miu[j] = relu(W3 @ x[:, j] + b3 + 2*b4 + W4 @ up[j-1] + W4 @ dn[j+1])

Implementation notes:
  - Data-parallel over batch: 16 rows -> 2 rows on each of 8 NeuronCores.
  - g is computed ONCE (K=5 fold matmul with rhs [x; ones]) and stored in
    SBUF bf16. Per step, 4 of every 8 PSUM banks ("A-banks") hold the pure
    W2 product and evacuate as tmp = psum + g (DVE tensor_tensor add, the
    only engine that can both touch PSUM and add: Act has no tensor_tensor
    and GPSIMD cannot access PSUM on this target) followed by
    state = relu(tmp) on Act/DVE (all-SBUF bf16). The other 4 ("F-banks")
    fold g in-PSUM on the PE as in the classic scheme and evacuate with
    one Act relu. The F/A ratio balances PE time (64+16 matmuls/step)
    against measured engine rates (Act ~700ns, DVE ~620ns per pass; Pool
    is ~5.8us/pass on this silicon and must not be on the critical path).
    (Rejected alternates: fp8 DoubleRow matmuls and engine-preloading g
    into PSUM with start=False accumulation both break on this
    toolchain/HW - the former crashes the runtime, the latter silently
    zeroes the preload at the first matmul.)
  - t=0 state is relu(g) for BOTH lanes: written once into the up tile;
    t=1's dn matmuls read the up tile at a +1 column offset (an extra
    trailing guard column keeps that read in-bounds and zero at j=L).
  - State is bf16 in [128, 2, token] tiles (dim1 = K half). All matmuls
    bf16 (1 cycle/row; fp8 DoubleRow crashes this toolchain's runtime).
  - Weights/x are pre-converted on the host; no on-device dtype passes.
  - Weight-stationary grouping: per (lane, ot) group of 8 chunk-banks the
    kt=0 matmuls share one LDWEIGHTS, kt=1 another (dedupe surgery below).
  - Output is written bf16 and upcast/transposed on the host.
"""

import sys

sys.path.insert(0, "/opt/trn_rl_repo")

import ml_dtypes
import numpy as np

import concourse.bass as bass
import concourse.bacc as bacc
import concourse.mybir as mybir
import concourse.tile as tile
from concourse.bass_utils import run_bass_kernel_spmd
from concourse.tile_rust import add_dep_helper

BS, DIMS, L, D, T = 16, 4, 2048, 256, 8
NCORES = 8
BSL = BS // NCORES          # batch rows per core
LP = L + 1                  # row span incl. one guard column
SFREE = BSL * LP + 1        # state tile free size (+1 tail guard column)
CH = 512                    # token chunk (one PSUM bank)
NCH = L // CH               # chunks per batch row
F32 = mybir.dt.float32
BF16 = mybir.dt.bfloat16
RELU = mybir.ActivationFunctionType.Relu
CHUNKS = [(b, c) for b in range(BSL) for c in range(NCH)]
# engine rotations. Measured real rates per 512-col pass: Act ~700ns,
# DVE ~620ns (maybe ~2x for all-SBUF bf16), Pool ~5.8us (unusable — Q7
# software loop, and barred from PSUM anyway). PSUM passes: Act/DVE only.
ENG_PAT = "ADADADAD".replace(" ", "")            # t0/final psum passes
RELU_PAT = "AADAADAD".replace(" ", "")           # relu tmp -> state (SBUF)
FOLD_BANKS = (0, 1)                              # F-banks per 8-bank group


def _dedupe_ldweights(nc):
    """Post-Tile BIR surgery: drop Ldweights that reload the identical
    weight AP already resident in the PE array (weight-stationary groups),
    carrying their sem waits onto the next PE instruction."""
    def ldkey(ins):
        a = ins.ins[0]
        return (a.memref if hasattr(a, "memref") else str(a),
                getattr(a, "offset", None), str(getattr(a, "ap", None)),
                str(getattr(a, "dtype", None)),
                getattr(ins, "perf_mode", None),
                getattr(ins, "is_transpose", None),
                str(getattr(ins, "tile_position", None)))
    n_drop = 0
    for f in nc.m.functions:
        for blk in f.blocks:
            out = []
            last = None
            pending = []
            for ins in blk.instructions:
                cn = ins.__class__.__name__
                eng = getattr(ins, "engine", None)
                if cn == "InstLdweights":
                    key = ldkey(ins)
                    si = ins.sync_info
                    has_upd = bool(si and si.on_update)
                    if key == last and not has_upd:
                        if si and si.on_wait:
                            pending.extend(list(si.on_wait))
                        n_drop += 1
                        continue
                    last = key
                    out.append(ins)
                else:
                    if eng is not None and str(eng) in ("EngineType.PE", "PE"):
                        if cn == "InstMatmult":
                            if getattr(ins, "is_transpose", None):
                                last = None
                            if pending:
                                ins.sync_info.on_wait = (
                                    list(ins.sync_info.on_wait) + pending)
                                pending = []
                        elif cn not in ("InstEventSemaphore", "InstDrain",
                                        "InstNop"):
                            last = None
                            if pending:
                                ins.sync_info.on_wait = (
                                    list(ins.sync_info.on_wait) + pending)
                                pending = []
                    out.append(ins)
            assert not pending
            blk.instructions = out
    return n_drop


def _build_nc():
    nc = bacc.Bacc("TRN2", target_bir_lowering=False, debug=False,
                   num_devices=NCORES)

    xe_d = nc.dram_tensor("xe", [BSL, 5, L], BF16, kind="ExternalInput").ap()
    w2t_d = nc.dram_tensor("w2t", [128, 2, D], BF16,
                           kind="ExternalInput").ap()
    w4t_d = nc.dram_tensor("w4t", [128, 2, D], BF16,
                           kind="ExternalInput").ap()
    fs_d = nc.dram_tensor("folds", [5, D], BF16, kind="ExternalInput").ap()
    ff_d = nc.dram_tensor("foldf", [5, D], BF16, kind="ExternalInput").ap()
    out_d = nc.dram_tensor("out_loc", [BSL, D, L], BF16,
                           kind="ExternalOutput").ap()

    _prev_mm = [None]

    def _mm(*a, **kw):
        inst = nc.tensor.matmul(*a, **kw)
        if _prev_mm[0] is not None:
            add_dep_helper(inst.ins, _prev_mm[0], sync=False,
                           reason="pin PE weight-stationary order")
        _prev_mm[0] = inst.ins
        return inst

    _eng_i = [0]

    def _rot():
        e = ENG_PAT[_eng_i[0] % len(ENG_PAT)]
        _eng_i[0] += 1
        return e

    def copy_pass(dst, src):
        e = _rot()
        if e == "A":
            nc.scalar.copy(dst, src)
        elif e == "D":
            nc.vector.tensor_copy(dst, src)
        else:
            nc.gpsimd.tensor_copy(dst, src)

    def relu_pass(dst, src):
        e = _rot()
        if e == "A":
            nc.scalar.activation(dst, src, RELU)
        elif e == "D":
            nc.vector.tensor_scalar_max(dst, src, 0.0)
        else:
            nc.gpsimd.tensor_scalar_max(dst, src, 0.0)

    _relu_i = [0]

    def relu2_pass(dst, src):
        e = RELU_PAT[_relu_i[0] % len(RELU_PAT)]
        _relu_i[0] += 1
        if e == "A":
            nc.scalar.activation(dst, src, RELU)
        elif e == "D":
            nc.vector.tensor_scalar_max(dst, src, 0.0)
        else:
            nc.gpsimd.tensor_scalar_max(dst, src, 0.0)

    with tile.TileContext(nc) as tc:
        with (
            tc.tile_pool(name="const", bufs=1) as cpool,
            tc.tile_pool(name="state", bufs=1) as spool,
            tc.tile_pool(name="stage", bufs=4) as stpool,
            tc.tile_pool(name="tmp", bufs=8) as tpool,
            tc.tile_pool(name="psum", bufs=8, space="PSUM") as ppool,
        ):
            # ------- PE warm-up: dummy matmuls with no input deps keep the
            # array busy (p-state ramp) while the first DMAs land.
            wsrc = cpool.tile([128, CH], BF16, name="wsrc")
            nc.vector.memset(wsrc[:, :], 0.0)
            for _ in range(12):
                wpt = ppool.tile([128, CH], F32, name="mm")
                _mm(wpt, wsrc[:, 0:128], wsrc[:, :], start=True, stop=True)

            # ------- t=0-critical loads first: fold_s + x rhs
            fold_s = cpool.tile([5, D], BF16, name="fold_s")
            nc.sync.dma_start(fold_s[0:5, :], fs_d[:, :])
            rhs5 = spool.tile([5, BSL * L], BF16, name="rhs5")
            for b in range(BSL):
                nc.sync.dma_start(rhs5[0:5, b * L:(b + 1) * L], xe_d[b])

            # ------- remaining weights (needed from t=1 / final)
            w2q = cpool.tile([128, 2, D], BF16, name="w2q")
            nc.scalar.dma_start(w2q[:, :, :], w2t_d[:, :, :])
            w4q = cpool.tile([128, 2, D], BF16, name="w4q")
            nc.scalar.dma_start(w4q[:, :, :], w4t_d[:, :, :])
            fold_f = cpool.tile([5, D], BF16, name="fold_f")
            nc.scalar.dma_start(fold_f[0:5, :], ff_d[:, :])

            # ------- state buffers + g
            # up token l of row b -> column b*LP + 1 + l (guard at b*LP,
            # tail guard at 2*LP); dn token l -> column b*LP + l (guard at
            # b*LP + L).
            up = [spool.tile([128, 2, SFREE], BF16, name=f"up{pp}")
                  for pp in range(2)]
            dn = [spool.tile([128, 2, SFREE], BF16, name=f"dn{pp}")
                  for pp in range(2)]
            gbuf = spool.tile([128, 2, BSL * L], BF16, name="gbuf")
            for t_ in (up[0], up[1]):
                for col in (0, LP, 2 * LP):
                    nc.vector.memset(t_[:, :, col:col + 1], 0.0)
            for t_ in (dn[0], dn[1]):
                for col in (L, L + LP, SFREE - 1):
                    nc.vector.memset(t_[:, :, col:col + 1], 0.0)

            # ------- t=0: g via fold; state0 = relu(g) (up tile only)
            for ot in range(2):
                pts = []
                for (b, c) in CHUNKS:
                    pt = ppool.tile([128, CH], F32, name="mm")
                    _mm(pt, fold_s[0:5, ot * 128:(ot + 1) * 128],
                        rhs5[0:5, b * L + c * CH: b * L + (c + 1) * CH],
                        start=True, stop=True)
                    pts.append(pt)
                for i, (b, c) in enumerate(CHUNKS):
                    tok = b * L + c * CH
                    wbase = b * LP + 1 + c * CH
                    copy_pass(gbuf[:, ot, tok: tok + CH], pts[i])
                    relu_pass(up[0][:, ot, wbase: wbase + CH], pts[i])

            # ------- T-1 recurrence steps (t=0 handled above)
            for t in range(1, T):
                dstp = t % 2
                srcp = (t + 1) % 2
                for lane in range(2):           # 0 = up, 1 = dn
                    sbuf = (up if lane == 0 else dn)[srcp]
                    roff = lane                 # dn reads token j+1
                    if t == 1:
                        sbuf = up[0]            # shared relu(g) state
                        roff = 2 * lane         # +1 col shift in up layout
                    dbuf = (up if lane == 0 else dn)[dstp]
                    for ot in range(2):
                        pts = [ppool.tile([128, CH], F32, name="mm")
                               for _ in CHUNKS]
                        for i, (b, c) in enumerate(CHUNKS):
                            if i in FOLD_BANKS:
                                _mm(pts[i],
                                    fold_s[0:5, ot * 128:(ot + 1) * 128],
                                    rhs5[0:5, b * L + c * CH:
                                         b * L + (c + 1) * CH],
                                    start=True, stop=False)
                        for kt in range(2):
                            for i, (b, c) in enumerate(CHUNKS):
                                base = b * LP + c * CH + roff
                                _mm(pts[i],
                                    w2q[:, kt, ot * 128:(ot + 1) * 128],
                                    sbuf[:, kt, base: base + CH],
                                    start=(kt == 0 and i not in FOLD_BANKS),
                                    stop=(kt == 1))
                        for i, (b, c) in enumerate(CHUNKS):
                            tok = b * L + c * CH
                            wbase = b * LP + c * CH + (1 - lane)
                            dst = dbuf[:, ot, wbase: wbase + CH]
                            if i in FOLD_BANKS:
                                nc.scalar.activation(dst, pts[i], RELU)
                            else:
                                tmp = tpool.tile([128, CH], BF16, name="tmp")
                                nc.vector.tensor_add(
                                    tmp, pts[i], gbuf[:, ot, tok: tok + CH])
                                relu2_pass(dst, tmp)

            # ------- final miu = relu(c + W4 up[j-1] + W4 dn[j+1])
            fsrc = (T - 1) % 2
            for ot in range(2):
                pts = []
                for (b, c) in CHUNKS:
                    pt = ppool.tile([128, CH], F32, name="mm")
                    _mm(pt, fold_f[0:5, ot * 128:(ot + 1) * 128],
                        rhs5[0:5, b * L + c * CH: b * L + (c + 1) * CH],
                        start=True, stop=False)
                    pts.append(pt)
                for kt in range(2):
                    for lane, fbuf in ((0, up[fsrc]), (1, dn[fsrc])):
                        for i, (b, c) in enumerate(CHUNKS):
                            base = b * LP + c * CH + lane
                            _mm(pts[i],
                                w4q[:, kt, ot * 128:(ot + 1) * 128],
                                fbuf[:, kt, base: base + CH],
                                start=False, stop=(kt == 1 and lane == 1))
                for i, (b, c) in enumerate(CHUNKS):
                    st = stpool.tile([128, CH], BF16, name="ostage")
                    relu_pass(st, pts[i])
                    if i % 2 == 0:
                        nc.sync.dma_start(
                            out_d[b, ot * 128:(ot + 1) * 128,
                                  c * CH:(c + 1) * CH], st)
                    else:
                        nc.scalar.dma_start(
                            out_d[b, ot * 128:(ot + 1) * 128,
                                  c * CH:(c + 1) * CH], st)
    _dedupe_ldweights(nc)
    # Excess matmul waits are split into EventSemaphore instructions by
    # generate_event_semaphores; moving them onto (now shared) Ldweights
    # would be wrong.
    nc.move_matmul_waits_to_ldweights = lambda: None
    nc.compile()
    return nc


_NC_CACHE = None


def _get_nc():
    global _NC_CACHE
    if _NC_CACHE is None:
        _NC_CACHE = _build_nc()
    return _NC_CACHE


def _prep_host(inputs):
    """Host-side weight preprocessing -> per-core in_maps."""
    f = np.float32
    bf = ml_dtypes.bfloat16
    x = np.ascontiguousarray(inputs["x"], dtype=f)          # (16, 4, 2048)
    W1, b1 = inputs["W1"].astype(f), inputs["b1"].astype(f)
    W2, b2 = inputs["W2"].astype(f), inputs["b2"].astype(f)
    W3, b3 = inputs["W3"].astype(f), inputs["b3"].astype(f)
    W4, b4 = inputs["W4"].astype(f), inputs["b4"].astype(f)
    # [p, kt, m] = W.T[kt*128 + p, m]
    w2t = np.ascontiguousarray(
        W2.T.reshape(2, 128, D).transpose(1, 0, 2)).astype(bf)
    w4t = np.ascontiguousarray(
        W4.T.reshape(2, 128, D).transpose(1, 0, 2)).astype(bf)
    folds = np.concatenate([W1.T, (b1 + b2)[None, :]], axis=0).astype(bf)
    foldf = np.concatenate([W3.T, (b3 + 2.0 * b4)[None, :]],
                           axis=0).astype(bf)
    ones = np.ones((BSL, 1, L), dtype=f)
    in_maps = []
    for c in range(NCORES):
        xe = np.ascontiguousarray(np.concatenate(
            [x[c * BSL:(c + 1) * BSL], ones], axis=1)).astype(bf)
        in_maps.append(dict(xe=xe, w2t=w2t, w4t=w4t,
                            folds=folds, foldf=foldf))
    return in_maps


def _run(inputs, trace=False):
    nc = _get_nc()
    in_maps = _prep_host(inputs)
    res = run_bass_kernel_spmd(nc, in_maps, core_ids=list(range(NCORES)),
                               trace=trace)
    parts = [res.results[c]["out_loc"] for c in range(NCORES)]
    full = np.concatenate(parts, axis=0).astype(np.float32)  # (16, 256, 2048)
    out = np.ascontiguousarray(full.transpose(0, 2, 1))     # (16, 2048, 256)
    return out, res


def kernel(**inputs):
    out, _ = _run(inputs, trace=False)
    return out


if __name__ == "__main__":
    nc = _build_nc()
    print("build ok")


# revision 22
# speedup vs baseline: 1.1904x; 1.1904x over previous
"""Trainium2 Bass kernel for the Bahdanau-style band recurrence.

Math (per batch row b, position j, T=8 steps):
    g[j]   = W1 @ x[:, j] + b1 + b2                      (d=256)
    up[j]  <- relu(g[j] + W2 @ up[j-1])   (up[-1] = 0)
    dn[j]  <- relu(g[j] + W2 @ dn[j+1])   (dn[L]  = 0)
    miu[j] = relu(W3 @ x[:, j] + b3 + 2*b4 + W4 @ up[j-1] + W4 @ dn[j+1])

Implementation notes:
  - Data-parallel over batch: 16 rows -> 2 rows on each of 8 NeuronCores.
  - The affine g-term is folded into each step's PSUM accumulation as a
    K=5 matmul with rhs [x; ones] and lhsT [W1^T; b1+b2]. Fold inputs are
    replicated at partition offsets 0/32/64/96 and the folds issued at
    tile_position (32*q, 0): on real HW the four row-band tiles execute
    concurrently (measured ~5ns median per extra tiled fold), so the g
    injection is ~4x cheaper than a full-height matmul per bank. Measured
    alternates that LOSE to this on silicon: evacuation-side g-adds (DVE
    tensor_tensor) poison the pipeline with cross-engine waits (~+90ns on
    every W2 matmul), GPSIMD passes take ~5.8us each, and fp8 DoubleRow /
    start=False-accumulate-on-preload are broken in this toolchain.
  - t=0 state is relu(g) for BOTH lanes: computed once into the up tile;
    t=1's dn matmuls read the up tile at a +1 column offset (an extra
    trailing guard column keeps that read in-bounds and zero at j=L).
  - State is bf16 in [128, 2, token] tiles (dim1 = K half). Weights and
    x are pre-converted to bf16 on the host - no on-device cast passes.
  - Per-step relu evacuations alternate DVE / Act (~720ns per 512-col
    pass each, well under the PE step time).
  - Output is written bf16 and upcast/transposed on the host.
"""

import sys

sys.path.insert(0, "/opt/trn_rl_repo")

import ml_dtypes
import numpy as np

import concourse.bass as bass
import concourse.bacc as bacc
import concourse.mybir as mybir
import concourse.tile as tile
from concourse.bass_utils import run_bass_kernel_spmd
from concourse.tile_rust import add_dep_helper

BS, DIMS, L, D, T = 16, 4, 2048, 256, 8
NCORES = 8
BSL = BS // NCORES          # batch rows per core
LP = L + 1                  # row span incl. one guard column
SFREE = BSL * LP + 1        # state tile free size (+1 tail guard column)
CH = 512                    # token chunk (one PSUM bank)
NCH = L // CH               # chunks per batch row
F32 = mybir.dt.float32
BF16 = mybir.dt.bfloat16
RELU = mybir.ActivationFunctionType.Relu
CHUNKS = [(b, c) for b in range(BSL) for c in range(NCH)]


def _dedupe_ldweights(nc):
    """Post-Tile BIR surgery: drop Ldweights that reload the identical
    weight AP already resident in the PE array (weight-stationary groups),
    carrying their sem waits onto the next PE instruction."""
    def ldkey(ins):
        a = ins.ins[0]
        return (a.memref if hasattr(a, "memref") else str(a),
                getattr(a, "offset", None), str(getattr(a, "ap", None)),
                str(getattr(a, "dtype", None)),
                getattr(ins, "perf_mode", None),
                getattr(ins, "is_transpose", None),
                str(getattr(ins, "tile_position", None)))
    n_drop = 0
    for f in nc.m.functions:
        for blk in f.blocks:
            out = []
            last = None
            pending = []
            for ins in blk.instructions:
                cn = ins.__class__.__name__
                eng = getattr(ins, "engine", None)
                if cn == "InstLdweights":
                    key = ldkey(ins)
                    si = ins.sync_info
                    has_upd = bool(si and si.on_update)
                    if key == last and not has_upd:
                        if si and si.on_wait:
                            pending.extend(list(si.on_wait))
                        n_drop += 1
                        continue
                    last = key
                    out.append(ins)
                else:
                    if eng is not None and str(eng) in ("EngineType.PE", "PE"):
                        if cn == "InstMatmult":
                            if getattr(ins, "is_transpose", None):
                                last = None
                            if pending:
                                ins.sync_info.on_wait = (
                                    list(ins.sync_info.on_wait) + pending)
                                pending = []
                        elif cn not in ("InstEventSemaphore", "InstDrain",
                                        "InstNop"):
                            last = None
                            if pending:
                                ins.sync_info.on_wait = (
                                    list(ins.sync_info.on_wait) + pending)
                                pending = []
                    out.append(ins)
            assert not pending
            blk.instructions = out
    return n_drop


def _build_nc():
    nc = bacc.Bacc("TRN2", target_bir_lowering=False, debug=False,
                   num_devices=NCORES)

    xe_d = nc.dram_tensor("xe", [BSL, 5, L], BF16, kind="ExternalInput").ap()
    w2t_d = nc.dram_tensor("w2t", [128, 2, D], BF16,
                           kind="ExternalInput").ap()
    w4t_d = nc.dram_tensor("w4t", [128, 2, D], BF16,
                           kind="ExternalInput").ap()
    fs_d = nc.dram_tensor("folds", [5, D], BF16, kind="ExternalInput").ap()
    ff_d = nc.dram_tensor("foldf", [5, D], BF16, kind="ExternalInput").ap()
    out_d = nc.dram_tensor("out_loc", [BSL, D, L], BF16,
                           kind="ExternalOutput").ap()

    _prev_mm = [None]

    def _mm(*a, **kw):
        inst = nc.tensor.matmul(*a, **kw)
        if _prev_mm[0] is not None:
            add_dep_helper(inst.ins, _prev_mm[0], sync=False,
                           reason="pin PE weight-stationary order")
        _prev_mm[0] = inst.ins
        return inst

    with tile.TileContext(nc) as tc:
        with (
            tc.tile_pool(name="const", bufs=1) as cpool,
            tc.tile_pool(name="state", bufs=1) as spool,
            tc.tile_pool(name="stage", bufs=4) as stpool,
            tc.tile_pool(name="psum", bufs=8, space="PSUM") as ppool,
        ):
            # ------- PE warm-up: dummy matmuls with no input deps keep the
            # array busy (p-state ramp) while the first DMAs land.
            wsrc = cpool.tile([128, CH], BF16, name="wsrc")
            nc.vector.memset(wsrc[:, :], 0.0)
            for _ in range(12):
                wpt = ppool.tile([128, CH], F32, name="mm")
                _mm(wpt, wsrc[:, 0:128], wsrc[:, :], start=True, stop=True)

            # ------- t=0-critical loads first: fold_s + x rhs, replicated
            # at partition offsets 0/32/64/96 for 4-wide tiled folds.
            fold_s = cpool.tile([128, D], BF16, name="fold_s")
            nc.sync.dma_start(fold_s[0:5, :], fs_d[:, :])
            rhs5 = spool.tile([128, BSL * L], BF16, name="rhs5")
            for b in range(BSL):
                nc.sync.dma_start(rhs5[0:5, b * L:(b + 1) * L], xe_d[b])
            for g in range(1, 4):
                nc.gpsimd.dma_start(rhs5[32 * g: 32 * g + 5, :], rhs5[0:5, :])
                nc.gpsimd.dma_start(fold_s[32 * g: 32 * g + 5, :],
                                    fold_s[0:5, :])

            # ------- remaining weights (needed from t=1 / final)
            w2q = cpool.tile([128, 2, D], BF16, name="w2q")
            nc.scalar.dma_start(w2q[:, :, :], w2t_d[:, :, :])
            w4q = cpool.tile([128, 2, D], BF16, name="w4q")
            nc.scalar.dma_start(w4q[:, :, :], w4t_d[:, :, :])
            fold_f = cpool.tile([128, D], BF16, name="fold_f")
            nc.scalar.dma_start(fold_f[0:5, :], ff_d[:, :])
            for g in range(1, 4):
                nc.gpsimd.dma_start(fold_f[32 * g: 32 * g + 5, :],
                                    fold_f[0:5, :])

            # ------- state buffers
            # up token l of row b -> column b*LP + 1 + l (guard at b*LP,
            # tail guard at 2*LP); dn token l -> column b*LP + l (guard at
            # b*LP + L).
            up = [spool.tile([128, 2, SFREE], BF16, name=f"up{pp}")
                  for pp in range(2)]
            dn = [spool.tile([128, 2, SFREE], BF16, name=f"dn{pp}")
                  for pp in range(2)]
            for t_ in (up[0], up[1]):
                for col in (0, LP, 2 * LP):
                    nc.vector.memset(t_[:, :, col:col + 1], 0.0)
            for t_ in (dn[0], dn[1]):
                for col in (L, L + LP, SFREE - 1):
                    nc.vector.memset(t_[:, :, col:col + 1], 0.0)

            def fold_mm(pt, i, lhs, ot, b, c, **kw):
                q = i % 4
                _mm(pt, lhs[32 * q: 32 * q + 5, ot * 128:(ot + 1) * 128],
                    rhs5[32 * q: 32 * q + 5,
                         b * L + c * CH: b * L + (c + 1) * CH],
                    tile_position=(32 * q, 0), **kw)

            def relu_evac(i, dst, src):
                if i % 2 == 0:
                    nc.vector.tensor_scalar_max(dst, src, 0.0)
                else:
                    nc.scalar.activation(dst, src, RELU)

            # ------- t=0: state0 = relu(g), written once (up tile only)
            for ot in range(2):
                pts = [ppool.tile([128, CH], F32, name="mm")
                       for _ in CHUNKS]
                for i, (b, c) in enumerate(CHUNKS):
                    fold_mm(pts[i], i, fold_s, ot, b, c,
                            start=True, stop=True)
                for i, (b, c) in enumerate(CHUNKS):
                    wbase = b * LP + 1 + c * CH
                    relu_evac(i + ot, up[0][:, ot, wbase: wbase + CH],
                              pts[i])

            # ------- T-1 recurrence steps (t=0 handled above)
            for t in range(1, T):
                dstp = t % 2
                srcp = (t + 1) % 2
                for lane in range(2):           # 0 = up, 1 = dn
                    sbuf = (up if lane == 0 else dn)[srcp]
                    roff = lane                 # dn reads token j+1
                    if t == 1:
                        sbuf = up[0]            # shared relu(g) state
                        roff = 2 * lane         # +1 col shift in up layout
                    dbuf = (up if lane == 0 else dn)[dstp]
                    for ot in range(2):
                        pts = [ppool.tile([128, CH], F32, name="mm")
                               for _ in CHUNKS]
                        for i, (b, c) in enumerate(CHUNKS):
                            fold_mm(pts[i], i, fold_s, ot, b, c,
                                    start=True, stop=False)
                        for kt in range(2):
                            for i, (b, c) in enumerate(CHUNKS):
                                base = b * LP + c * CH + roff
                                _mm(pts[i],
                                    w2q[:, kt, ot * 128:(ot + 1) * 128],
                                    sbuf[:, kt, base: base + CH],
                                    start=False, stop=(kt == 1))
                        for i, (b, c) in enumerate(CHUNKS):
                            wbase = b * LP + c * CH + (1 - lane)
                            relu_evac(i + ot + lane,
                                      dbuf[:, ot, wbase: wbase + CH],
                                      pts[i])

            # ------- final miu = relu(c + W4 up[j-1] + W4 dn[j+1])
            fsrc = (T - 1) % 2
            for ot in range(2):
                pts = [ppool.tile([128, CH], F32, name="mm")
                       for _ in CHUNKS]
                for i, (b, c) in enumerate(CHUNKS):
                    fold_mm(pts[i], i, fold_f, ot, b, c,
                            start=True, stop=False)
                for kt in range(2):
                    for lane, fbuf in ((0, up[fsrc]), (1, dn[fsrc])):
                        for i, (b, c) in enumerate(CHUNKS):
                            base = b * LP + c * CH + lane
                            _mm(pts[i],
                                w4q[:, kt, ot * 128:(ot + 1) * 128],
                                fbuf[:, kt, base: base + CH],
                                start=False, stop=(kt == 1 and lane == 1))
                for i, (b, c) in enumerate(CHUNKS):
                    st = stpool.tile([128, CH], BF16, name="ostage")
                    relu_evac(i + ot, st, pts[i])
                    if i % 2 == 0:
                        nc.sync.dma_start(
                            out_d[b, ot * 128:(ot + 1) * 128,
                                  c * CH:(c + 1) * CH], st)
                    else:
                        nc.scalar.dma_start(
                            out_d[b, ot * 128:(ot + 1) * 128,
                                  c * CH:(c + 1) * CH], st)
    _dedupe_ldweights(nc)
    # Excess matmul waits are split into EventSemaphore instructions by
    # generate_event_semaphores; moving them onto (now shared) Ldweights
    # would be wrong.
    nc.move_matmul_waits_to_ldweights = lambda: None
    nc.compile()
    return nc


_NC_CACHE = None


def _get_nc():
    global _NC_CACHE
    if _NC_CACHE is None:
        _NC_CACHE = _build_nc()
    return _NC_CACHE


def _prep_host(inputs):
    """Host-side weight preprocessing -> per-core in_maps."""
    f = np.float32
    bf = ml_dtypes.bfloat16
    x = np.ascontiguousarray(inputs["x"], dtype=f)          # (16, 4, 2048)
    W1, b1 = inputs["W1"].astype(f), inputs["b1"].astype(f)
    W2, b2 = inputs["W2"].astype(f), inputs["b2"].astype(f)
    W3, b3 = inputs["W3"].astype(f), inputs["b3"].astype(f)
    W4, b4 = inputs["W4"].astype(f), inputs["b4"].astype(f)
    # [p, kt, m] = W.T[kt*128 + p, m]
    w2t = np.ascontiguousarray(
        W2.T.reshape(2, 128, D).transpose(1, 0, 2)).astype(bf)
    w4t = np.ascontiguousarray(
        W4.T.reshape(2, 128, D).transpose(1, 0, 2)).astype(bf)
    folds = np.concatenate([W1.T, (b1 + b2)[None, :]], axis=0).astype(bf)
    foldf = np.concatenate([W3.T, (b3 + 2.0 * b4)[None, :]],
                           axis=0).astype(bf)
    ones = np.ones((BSL, 1, L), dtype=f)
    in_maps = []
    for c in range(NCORES):
        xe = np.ascontiguousarray(np.concatenate(
            [x[c * BSL:(c + 1) * BSL], ones], axis=1)).astype(bf)
        in_maps.append(dict(xe=xe, w2t=w2t, w4t=w4t,
                            folds=folds, foldf=foldf))
    return in_maps


def _run(inputs, trace=False):
    nc = _get_nc()
    in_maps = _prep_host(inputs)
    res = run_bass_kernel_spmd(nc, in_maps, core_ids=list(range(NCORES)),
                               trace=trace)
    parts = [res.results[c]["out_loc"] for c in range(NCORES)]
    full = np.concatenate(parts, axis=0).astype(np.float32)  # (16, 256, 2048)
    out = np.ascontiguousarray(full.transpose(0, 2, 1))     # (16, 2048, 256)
    return out, res


def kernel(**inputs):
    out, _ = _run(inputs, trace=False)
    return out


if __name__ == "__main__":
    nc = _build_nc()
    print("build ok")


# revision 25
# speedup vs baseline: 1.1938x; 1.0029x over previous
"""Trainium2 Bass kernel for the Bahdanau-style band recurrence.

Math (per batch row b, position j, T=8 steps):
    g[j]   = W1 @ x[:, j] + b1 + b2                      (d=256)
    up[j]  <- relu(g[j] + W2 @ up[j-1])   (up[-1] = 0)
    dn[j]  <- relu(g[j] + W2 @ dn[j+1])   (dn[L]  = 0)
    miu[j] = relu(W3 @ x[:, j] + b3 + 2*b4 + W4 @ up[j-1] + W4 @ dn[j+1])

Implementation notes:
  - Data-parallel over batch: 16 rows -> 2 rows on each of 8 NeuronCores.
  - The affine g-term is folded into each step's PSUM accumulation as a
    K=5 matmul with rhs [x; ones] and lhsT [W1^T; b1+b2]. Fold inputs are
    replicated at partition offsets 0/32/64/96 and the folds issued at
    tile_position (32*q, 0): on real HW the four row-band tiles execute
    concurrently (measured ~5ns median per extra tiled fold), so the g
    injection is ~4x cheaper than a full-height matmul per bank. Measured
    alternates that LOSE to this on silicon: evacuation-side g-adds (DVE
    tensor_tensor) poison the pipeline with cross-engine waits (~+90ns on
    every W2 matmul), GPSIMD passes take ~5.8us each, and fp8 DoubleRow /
    start=False-accumulate-on-preload are broken in this toolchain.
  - t=0 state is relu(g) for BOTH lanes: computed once into the up tile;
    t=1's dn matmuls read the up tile at a +1 column offset (an extra
    trailing guard column keeps that read in-bounds and zero at j=L).
  - State is bf16 in [128, 2, token] tiles (dim1 = K half). Weights and
    x are pre-converted to bf16 on the host - no on-device cast passes.
  - Per-step relu evacuations alternate DVE / Act (~720ns per 512-col
    pass each, well under the PE step time).
  - Output is written bf16 and upcast/transposed on the host.
"""

import sys

sys.path.insert(0, "/opt/trn_rl_repo")

import ml_dtypes
import numpy as np

import concourse.bass as bass
import concourse.bacc as bacc
import concourse.mybir as mybir
import concourse.tile as tile
from concourse.bass_utils import run_bass_kernel_spmd
from concourse.tile_rust import add_dep_helper

BS, DIMS, L, D, T = 16, 4, 2048, 256, 8
NCORES = 8
BSL = BS // NCORES          # batch rows per core
LP = L + 1                  # row span incl. one guard column
SFREE = BSL * LP + 1        # state tile free size (+1 tail guard column)
CH = 512                    # token chunk (one PSUM bank)
NCH = L // CH               # chunks per batch row
F32 = mybir.dt.float32
BF16 = mybir.dt.bfloat16
RELU = mybir.ActivationFunctionType.Relu
CHUNKS = [(b, c) for b in range(BSL) for c in range(NCH)]


def _dedupe_ldweights(nc):
    """Post-Tile BIR surgery: drop Ldweights that reload the identical
    weight AP already resident in the PE array (weight-stationary groups),
    carrying their sem waits onto the next PE instruction."""
    def ldkey(ins):
        a = ins.ins[0]
        return (a.memref if hasattr(a, "memref") else str(a),
                getattr(a, "offset", None), str(getattr(a, "ap", None)),
                str(getattr(a, "dtype", None)),
                getattr(ins, "perf_mode", None),
                getattr(ins, "is_transpose", None),
                str(getattr(ins, "tile_position", None)))
    n_drop = 0
    for f in nc.m.functions:
        for blk in f.blocks:
            out = []
            last = None
            pending = []
            for ins in blk.instructions:
                cn = ins.__class__.__name__
                eng = getattr(ins, "engine", None)
                if cn == "InstLdweights":
                    key = ldkey(ins)
                    si = ins.sync_info
                    has_upd = bool(si and si.on_update)
                    if key == last and not has_upd:
                        if si and si.on_wait:
                            pending.extend(list(si.on_wait))
                        n_drop += 1
                        continue
                    last = key
                    out.append(ins)
                else:
                    if eng is not None and str(eng) in ("EngineType.PE", "PE"):
                        if cn == "InstMatmult":
                            if getattr(ins, "is_transpose", None):
                                last = None
                            if pending:
                                ins.sync_info.on_wait = (
                                    list(ins.sync_info.on_wait) + pending)
                                pending = []
                        elif cn not in ("InstEventSemaphore", "InstDrain",
                                        "InstNop"):
                            last = None
                            if pending:
                                ins.sync_info.on_wait = (
                                    list(ins.sync_info.on_wait) + pending)
                                pending = []
                    out.append(ins)
            assert not pending
            blk.instructions = out
    return n_drop


def _build_nc():
    nc = bacc.Bacc("TRN2", target_bir_lowering=False, debug=False,
                   num_devices=NCORES)

    xe_d = nc.dram_tensor("xe", [BSL, 5, L], BF16, kind="ExternalInput").ap()
    w2t_d = nc.dram_tensor("w2t", [128, 2, D], BF16,
                           kind="ExternalInput").ap()
    w4t_d = nc.dram_tensor("w4t", [128, 2, D], BF16,
                           kind="ExternalInput").ap()
    fs_d = nc.dram_tensor("folds", [5, D], BF16, kind="ExternalInput").ap()
    ff_d = nc.dram_tensor("foldf", [5, D], BF16, kind="ExternalInput").ap()
    out_d = nc.dram_tensor("out_loc", [BSL, D, L], BF16,
                           kind="ExternalOutput").ap()

    _prev_mm = [None]

    def _mm(*a, **kw):
        inst = nc.tensor.matmul(*a, **kw)
        if _prev_mm[0] is not None:
            add_dep_helper(inst.ins, _prev_mm[0], sync=False,
                           reason="pin PE weight-stationary order")
        _prev_mm[0] = inst.ins
        return inst

    with tile.TileContext(nc) as tc:
        with (
            tc.tile_pool(name="const", bufs=1) as cpool,
            tc.tile_pool(name="state", bufs=1) as spool,
            tc.tile_pool(name="stage", bufs=4) as stpool,
            tc.tile_pool(name="psum", bufs=8, space="PSUM") as ppool,
        ):
            # ------- PE warm-up: dummy matmuls with no input deps keep the
            # array busy (p-state ramp) while the first DMAs land.
            wsrc = cpool.tile([128, CH], BF16, name="wsrc")
            nc.vector.memset(wsrc[:, :], 0.0)
            for _ in range(12):
                wpt = ppool.tile([128, CH], F32, name="mm")
                _mm(wpt, wsrc[:, 0:128], wsrc[:, :], start=True, stop=True)

            # ------- t=0-critical loads first: fold_s + x rhs, replicated
            # at partition offsets 0/32/64/96 for 4-wide tiled folds.
            fold_s = cpool.tile([128, D], BF16, name="fold_s")
            nc.sync.dma_start(fold_s[0:5, :], fs_d[:, :])
            rhs5 = spool.tile([128, BSL * L], BF16, name="rhs5")
            for b in range(BSL):
                nc.sync.dma_start(rhs5[0:5, b * L:(b + 1) * L], xe_d[b])
            for g in range(1, 4):
                nc.gpsimd.dma_start(rhs5[32 * g: 32 * g + 5, :], rhs5[0:5, :])
                nc.gpsimd.dma_start(fold_s[32 * g: 32 * g + 5, :],
                                    fold_s[0:5, :])

            # ------- remaining weights (needed from t=1 / final)
            w2q = cpool.tile([128, 2, D], BF16, name="w2q")
            nc.scalar.dma_start(w2q[:, :, :], w2t_d[:, :, :])
            w4q = cpool.tile([128, 2, D], BF16, name="w4q")
            nc.scalar.dma_start(w4q[:, :, :], w4t_d[:, :, :])
            fold_f = cpool.tile([128, D], BF16, name="fold_f")
            nc.scalar.dma_start(fold_f[0:5, :], ff_d[:, :])
            for g in range(1, 4):
                nc.gpsimd.dma_start(fold_f[32 * g: 32 * g + 5, :],
                                    fold_f[0:5, :])

            # ------- state buffers
            # up token l of row b -> column b*LP + 1 + l (guard at b*LP,
            # tail guard at 2*LP); dn token l -> column b*LP + l (guard at
            # b*LP + L).
            up = [spool.tile([128, 2, SFREE], BF16, name=f"up{pp}")
                  for pp in range(2)]
            dn = [spool.tile([128, 2, SFREE], BF16, name=f"dn{pp}")
                  for pp in range(2)]
            for t_ in (up[0], up[1]):
                for col in (0, LP, 2 * LP):
                    nc.vector.memset(t_[:, :, col:col + 1], 0.0)
            for t_ in (dn[0], dn[1]):
                for col in (L, L + LP, SFREE - 1):
                    nc.vector.memset(t_[:, :, col:col + 1], 0.0)

            def fold_mm(pt, i, lhs, ot, b, c, **kw):
                q = i % 4
                _mm(pt, lhs[32 * q: 32 * q + 5, ot * 128:(ot + 1) * 128],
                    rhs5[32 * q: 32 * q + 5,
                         b * L + c * CH: b * L + (c + 1) * CH],
                    tile_position=(32 * q, 0), **kw)

            def relu_evac(i, dst, src):
                if i % 2 == 0:
                    nc.vector.tensor_scalar_max(dst, src, 0.0)
                else:
                    nc.scalar.activation(dst, src, RELU)

            # ------- t=0: state0 = relu(g), written once (up tile only)
            for ot in range(2):
                for hx, half in enumerate((CHUNKS[0:4], CHUNKS[4:8])):
                    pts = [ppool.tile([128, CH], F32, name="mm")
                           for _ in half]
                    for i, (b, c) in enumerate(half):
                        fold_mm(pts[i], i, fold_s, ot, b, c,
                                start=True, stop=True)
                    for i, (b, c) in enumerate(half):
                        wbase = b * LP + 1 + c * CH
                        relu_evac(i + ot,
                                  up[0][:, ot, wbase: wbase + CH], pts[i])

            # ------- T-1 recurrence steps (t=0 handled above)
            for t in range(1, T):
                dstp = t % 2
                srcp = (t + 1) % 2
                for lane in range(2):           # 0 = up, 1 = dn
                    sbuf = (up if lane == 0 else dn)[srcp]
                    roff = lane                 # dn reads token j+1
                    if t == 1:
                        sbuf = up[0]            # shared relu(g) state
                        roff = 2 * lane         # +1 col shift in up layout
                    dbuf = (up if lane == 0 else dn)[dstp]
                    for ot in range(2):
                        for half in (CHUNKS[0:4], CHUNKS[4:8]):
                            pts = [ppool.tile([128, CH], F32, name="mm")
                                   for _ in half]
                            for i, (b, c) in enumerate(half):
                                fold_mm(pts[i], i, fold_s, ot, b, c,
                                        start=True, stop=False)
                            for kt in range(2):
                                for i, (b, c) in enumerate(half):
                                    base = b * LP + c * CH + roff
                                    _mm(pts[i],
                                        w2q[:, kt, ot * 128:(ot + 1) * 128],
                                        sbuf[:, kt, base: base + CH],
                                        start=False, stop=(kt == 1))
                            for i, (b, c) in enumerate(half):
                                wbase = b * LP + c * CH + (1 - lane)
                                relu_evac(i + ot + lane,
                                          dbuf[:, ot, wbase: wbase + CH],
                                          pts[i])

            # ------- final miu = relu(c + W4 up[j-1] + W4 dn[j+1])
            fsrc = (T - 1) % 2
            for ot in range(2):
                for half in (CHUNKS[0:4], CHUNKS[4:8]):
                    pts = [ppool.tile([128, CH], F32, name="mm")
                           for _ in half]
                    for i, (b, c) in enumerate(half):
                        fold_mm(pts[i], i, fold_f, ot, b, c,
                                start=True, stop=False)
                    for kt in range(2):
                        for lane, fbuf in ((0, up[fsrc]), (1, dn[fsrc])):
                            for i, (b, c) in enumerate(half):
                                base = b * LP + c * CH + lane
                                _mm(pts[i],
                                    w4q[:, kt, ot * 128:(ot + 1) * 128],
                                    fbuf[:, kt, base: base + CH],
                                    start=False,
                                    stop=(kt == 1 and lane == 1))
                    for i, (b, c) in enumerate(half):
                        st = stpool.tile([128, CH], BF16, name="ostage")
                        relu_evac(i + ot, st, pts[i])
                        if i % 2 == 0:
                            nc.sync.dma_start(
                                out_d[b, ot * 128:(ot + 1) * 128,
                                      c * CH:(c + 1) * CH], st)
                        else:
                            nc.scalar.dma_start(
                                out_d[b, ot * 128:(ot + 1) * 128,
                                      c * CH:(c + 1) * CH], st)
    _dedupe_ldweights(nc)
    # Excess matmul waits are split into EventSemaphore instructions by
    # generate_event_semaphores; moving them onto (now shared) Ldweights
    # would be wrong.
    nc.move_matmul_waits_to_ldweights = lambda: None
    nc.compile()
    return nc


_NC_CACHE = None


def _get_nc():
    global _NC_CACHE
    if _NC_CACHE is None:
        _NC_CACHE = _build_nc()
    return _NC_CACHE


def _prep_host(inputs):
    """Host-side weight preprocessing -> per-core in_maps."""
    f = np.float32
    bf = ml_dtypes.bfloat16
    x = np.ascontiguousarray(inputs["x"], dtype=f)          # (16, 4, 2048)
    W1, b1 = inputs["W1"].astype(f), inputs["b1"].astype(f)
    W2, b2 = inputs["W2"].astype(f), inputs["b2"].astype(f)
    W3, b3 = inputs["W3"].astype(f), inputs["b3"].astype(f)
    W4, b4 = inputs["W4"].astype(f), inputs["b4"].astype(f)
    # [p, kt, m] = W.T[kt*128 + p, m]
    w2t = np.ascontiguousarray(
        W2.T.reshape(2, 128, D).transpose(1, 0, 2)).astype(bf)
    w4t = np.ascontiguousarray(
        W4.T.reshape(2, 128, D).transpose(1, 0, 2)).astype(bf)
    folds = np.concatenate([W1.T, (b1 + b2)[None, :]], axis=0).astype(bf)
    foldf = np.concatenate([W3.T, (b3 + 2.0 * b4)[None, :]],
                           axis=0).astype(bf)
    ones = np.ones((BSL, 1, L), dtype=f)
    in_maps = []
    for c in range(NCORES):
        xe = np.ascontiguousarray(np.concatenate(
            [x[c * BSL:(c + 1) * BSL], ones], axis=1)).astype(bf)
        in_maps.append(dict(xe=xe, w2t=w2t, w4t=w4t,
                            folds=folds, foldf=foldf))
    return in_maps


def _run(inputs, trace=False):
    nc = _get_nc()
    in_maps = _prep_host(inputs)
    res = run_bass_kernel_spmd(nc, in_maps, core_ids=list(range(NCORES)),
                               trace=trace)
    parts = [res.results[c]["out_loc"] for c in range(NCORES)]
    full = np.concatenate(parts, axis=0).astype(np.float32)  # (16, 256, 2048)
    out = np.ascontiguousarray(full.transpose(0, 2, 1))     # (16, 2048, 256)
    return out, res


def kernel(**inputs):
    out, _ = _run(inputs, trace=False)
    return out


if __name__ == "__main__":
    nc = _build_nc()
    print("build ok")


# revision 27
# speedup vs baseline: 1.4321x; 1.1997x over previous
"""Trainium2 Bass kernel for the Bahdanau-style band recurrence.

Math (per batch row b, position j, T=8 steps):
    g[j]   = W1 @ x[:, j] + b1 + b2                      (d=256)
    up[j]  <- relu(g[j] + W2 @ up[j-1])   (up[-1] = 0)
    dn[j]  <- relu(g[j] + W2 @ dn[j+1])   (dn[L]  = 0)
    miu[j] = relu(W3 @ x[:, j] + b3 + 2*b4 + W4 @ up[j-1] + W4 @ dn[j+1])

Implementation notes:
  - Data-parallel over batch: 16 rows -> 2 rows on each of 8 NeuronCores.
  - The affine g-term is folded into each step's PSUM accumulation as a
    K=5 matmul with rhs [x; ones] and lhsT [W1^T; b1+b2]. Fold inputs are
    replicated at partition offsets 0/32/64/96 and the folds issued at
    tile_position (32*q, 0): on real HW the four row-band tiles execute
    concurrently (measured ~5ns median per extra tiled fold), so the g
    injection is ~4x cheaper than a full-height matmul per bank. Measured
    alternates that LOSE to this on silicon: evacuation-side g-adds (DVE
    tensor_tensor) poison the pipeline with cross-engine waits (~+90ns on
    every W2 matmul), GPSIMD passes take ~5.8us each, and fp8 DoubleRow /
    start=False-accumulate-on-preload are broken in this toolchain.
  - t=0 state is relu(g) for BOTH lanes: computed once into the up tile;
    t=1's dn matmuls read the up tile at a +1 column offset (an extra
    trailing guard column keeps that read in-bounds and zero at j=L).
  - State is bf16 in [128, 2, token] tiles (dim1 = K half). Weights and
    x are pre-converted to bf16 on the host - no on-device cast passes.
  - Per-step relu evacuations alternate DVE / Act (~720ns per 512-col
    pass each, well under the PE step time).
  - Output is written bf16 and upcast/transposed on the host.
"""

import sys

sys.path.insert(0, "/opt/trn_rl_repo")

import ml_dtypes
import numpy as np

import concourse.bass as bass
import concourse.bacc as bacc
import concourse.mybir as mybir
import concourse.tile as tile
from concourse.bass_utils import run_bass_kernel_spmd
from concourse.tile_rust import add_dep_helper

BS, DIMS, L, D, T = 16, 4, 2048, 256, 8
NCORES = 8
BSL = BS // NCORES          # batch rows per core
LP = L + 1                  # row span incl. one guard column
SFREE = BSL * LP + 1        # state tile free size (+1 tail guard column)
CH = 512                    # token chunk (one PSUM bank)
NCH = L // CH               # chunks per batch row
F32 = mybir.dt.float32
BF16 = mybir.dt.bfloat16
RELU = mybir.ActivationFunctionType.Relu
CHUNKS = [(b, c) for b in range(BSL) for c in range(NCH)]


def _dedupe_ldweights(nc):
    """Post-Tile BIR surgery: drop Ldweights that reload the identical
    weight AP already resident in the PE array (weight-stationary groups),
    carrying their sem waits onto the next PE instruction."""
    def ldkey(ins):
        a = ins.ins[0]
        return (a.memref if hasattr(a, "memref") else str(a),
                getattr(a, "offset", None), str(getattr(a, "ap", None)),
                str(getattr(a, "dtype", None)),
                getattr(ins, "perf_mode", None),
                getattr(ins, "is_transpose", None),
                str(getattr(ins, "tile_position", None)))
    n_drop = 0
    for f in nc.m.functions:
        for blk in f.blocks:
            out = []
            last = None
            pending = []
            for ins in blk.instructions:
                cn = ins.__class__.__name__
                eng = getattr(ins, "engine", None)
                if cn == "InstLdweights":
                    key = ldkey(ins)
                    si = ins.sync_info
                    has_upd = bool(si and si.on_update)
                    if key == last and not has_upd:
                        if si and si.on_wait:
                            pending.extend(list(si.on_wait))
                        n_drop += 1
                        continue
                    last = key
                    out.append(ins)
                else:
                    if eng is not None and str(eng) in ("EngineType.PE", "PE"):
                        if cn == "InstMatmult":
                            if getattr(ins, "is_transpose", None):
                                last = None
                            if pending:
                                ins.sync_info.on_wait = (
                                    list(ins.sync_info.on_wait) + pending)
                                pending = []
                        elif cn not in ("InstEventSemaphore", "InstDrain",
                                        "InstNop"):
                            last = None
                            if pending:
                                ins.sync_info.on_wait = (
                                    list(ins.sync_info.on_wait) + pending)
                                pending = []
                    out.append(ins)
            assert not pending
            blk.instructions = out
    return n_drop


def _build_nc():
    nc = bacc.Bacc("TRN2", target_bir_lowering=False, debug=False,
                   num_devices=NCORES)

    xe_d = nc.dram_tensor("xe", [BSL, 5, L], BF16, kind="ExternalInput").ap()
    w2t_d = nc.dram_tensor("w2t", [128, 2, D], BF16,
                           kind="ExternalInput").ap()
    w4t_d = nc.dram_tensor("w4t", [128, 2, D], BF16,
                           kind="ExternalInput").ap()
    fs_d = nc.dram_tensor("folds", [5, D], BF16, kind="ExternalInput").ap()
    ff_d = nc.dram_tensor("foldf", [5, D], BF16, kind="ExternalInput").ap()
    out_d = nc.dram_tensor("out_loc", [BSL, D, L], BF16,
                           kind="ExternalOutput").ap()

    _prev_mm = [None]

    def _mm(*a, **kw):
        inst = nc.tensor.matmul(*a, **kw)
        if _prev_mm[0] is not None:
            add_dep_helper(inst.ins, _prev_mm[0], sync=False,
                           reason="pin PE weight-stationary order")
        _prev_mm[0] = inst.ins
        return inst

    with tile.TileContext(nc) as tc:
        with (
            tc.tile_pool(name="const", bufs=1) as cpool,
            tc.tile_pool(name="state", bufs=1) as spool,
            tc.tile_pool(name="stage", bufs=4) as stpool,
            tc.tile_pool(name="psum", bufs=8, space="PSUM") as ppool,
        ):
            # ------- PE warm-up: dummy matmuls with no input deps keep the
            # array busy (p-state ramp) while the first DMAs land.
            wsrc = cpool.tile([128, CH], BF16, name="wsrc")
            nc.vector.memset(wsrc[:, :], 0.0)
            for _ in range(12):
                wpt = ppool.tile([128, CH], F32, name="mm")
                _mm(wpt, wsrc[:, 0:128], wsrc[:, :], start=True, stop=True)

            # ------- t=0-critical loads first: fold_s + x rhs, replicated
            # at partition offsets 0/32/64/96 for 4-wide tiled folds.
            fold_s = cpool.tile([128, D], BF16, name="fold_s")
            nc.sync.dma_start(fold_s[0:5, :], fs_d[:, :])
            rhs5 = spool.tile([128, BSL * L], BF16, name="rhs5")
            for b in range(BSL):
                nc.sync.dma_start(rhs5[0:5, b * L:(b + 1) * L], xe_d[b])
            for g, eng in ((1, nc.sync), (2, nc.scalar), (3, nc.gpsimd)):
                eng.dma_start(rhs5[32 * g: 32 * g + 5, :], rhs5[0:5, :])
                eng.dma_start(fold_s[32 * g: 32 * g + 5, :], fold_s[0:5, :])

            # ------- remaining weights (needed from t=1 / final)
            w2q = cpool.tile([128, 2, D], BF16, name="w2q")
            nc.scalar.dma_start(w2q[:, :, :], w2t_d[:, :, :])
            w4q = cpool.tile([128, 2, D], BF16, name="w4q")
            nc.scalar.dma_start(w4q[:, :, :], w4t_d[:, :, :])
            fold_f = cpool.tile([128, D], BF16, name="fold_f")
            nc.scalar.dma_start(fold_f[0:5, :], ff_d[:, :])
            for g in range(1, 4):
                nc.gpsimd.dma_start(fold_f[32 * g: 32 * g + 5, :],
                                    fold_f[0:5, :])

            # ------- state buffers
            # up token l of row b -> column b*LP + 1 + l (guard at b*LP,
            # tail guard at 2*LP); dn token l -> column b*LP + l (guard at
            # b*LP + L).
            up = [spool.tile([128, 2, SFREE], BF16, name=f"up{pp}")
                  for pp in range(2)]
            dn = [spool.tile([128, 2, SFREE], BF16, name=f"dn{pp}")
                  for pp in range(2)]
            for t_ in (up[0], up[1]):
                for col in (0, LP, 2 * LP):
                    nc.vector.memset(t_[:, :, col:col + 1], 0.0)
            for t_ in (dn[0], dn[1]):
                for col in (L, L + LP, SFREE - 1):
                    nc.vector.memset(t_[:, :, col:col + 1], 0.0)

            def fold_mm(pt, i, lhs, ot, b, c, **kw):
                q = i % 4
                _mm(pt, lhs[32 * q: 32 * q + 5, ot * 128:(ot + 1) * 128],
                    rhs5[32 * q: 32 * q + 5,
                         b * L + c * CH: b * L + (c + 1) * CH],
                    tile_position=(32 * q, 0), **kw)

            def relu_evac(i, dst, src):
                if i % 2 == 0:
                    nc.vector.tensor_scalar_max(dst, src, 0.0)
                else:
                    nc.scalar.activation(dst, src, RELU)

            # ------- t=0: state0 = relu(g), written once (up tile only)
            for ot in range(2):
                for hx, half in enumerate((CHUNKS[0:4], CHUNKS[4:8])):
                    pts = [ppool.tile([128, CH], F32, name="mm")
                           for _ in half]
                    for i, (b, c) in enumerate(half):
                        fold_mm(pts[i], i, fold_s, ot, b, c,
                                start=True, stop=True)
                    for i, (b, c) in enumerate(half):
                        wbase = b * LP + 1 + c * CH
                        relu_evac(i + ot,
                                  up[0][:, ot, wbase: wbase + CH], pts[i])

            # ------- T-1 recurrence steps (t=0 handled above)
            for t in range(1, T):
                dstp = t % 2
                srcp = (t + 1) % 2
                for lane in range(2):           # 0 = up, 1 = dn
                    sbuf = (up if lane == 0 else dn)[srcp]
                    roff = lane                 # dn reads token j+1
                    if t == 1:
                        sbuf = up[0]            # shared relu(g) state
                        roff = 2 * lane         # +1 col shift in up layout
                    dbuf = (up if lane == 0 else dn)[dstp]
                    for ot in range(2):
                        for half in (CHUNKS[0:4], CHUNKS[4:8]):
                            pts = [ppool.tile([128, CH], F32, name="mm")
                                   for _ in half]
                            for i, (b, c) in enumerate(half):
                                fold_mm(pts[i], i, fold_s, ot, b, c,
                                        start=True, stop=False)
                            for kt in range(2):
                                for i, (b, c) in enumerate(half):
                                    base = b * LP + c * CH + roff
                                    _mm(pts[i],
                                        w2q[:, kt, ot * 128:(ot + 1) * 128],
                                        sbuf[:, kt, base: base + CH],
                                        start=False, stop=(kt == 1))
                            for i, (b, c) in enumerate(half):
                                wbase = b * LP + c * CH + (1 - lane)
                                relu_evac(i + ot + lane,
                                          dbuf[:, ot, wbase: wbase + CH],
                                          pts[i])

            # ------- final miu = relu(c + W4 up[j-1] + W4 dn[j+1])
            fsrc = (T - 1) % 2
            for ot in range(2):
                for half in (CHUNKS[0:4], CHUNKS[4:8]):
                    pts = [ppool.tile([128, CH], F32, name="mm")
                           for _ in half]
                    for i, (b, c) in enumerate(half):
                        fold_mm(pts[i], i, fold_f, ot, b, c,
                                start=True, stop=False)
                    for kt in range(2):
                        for lane, fbuf in ((0, up[fsrc]), (1, dn[fsrc])):
                            for i, (b, c) in enumerate(half):
                                base = b * LP + c * CH + lane
                                _mm(pts[i],
                                    w4q[:, kt, ot * 128:(ot + 1) * 128],
                                    fbuf[:, kt, base: base + CH],
                                    start=False,
                                    stop=(kt == 1 and lane == 1))
                    for i, (b, c) in enumerate(half):
                        st = stpool.tile([128, CH], BF16, name="ostage")
                        relu_evac(i + ot, st, pts[i])
                        if i % 2 == 0:
                            nc.sync.dma_start(
                                out_d[b, ot * 128:(ot + 1) * 128,
                                      c * CH:(c + 1) * CH], st)
                        else:
                            nc.scalar.dma_start(
                                out_d[b, ot * 128:(ot + 1) * 128,
                                      c * CH:(c + 1) * CH], st)
    _dedupe_ldweights(nc)
    # Excess matmul waits are split into EventSemaphore instructions by
    # generate_event_semaphores; moving them onto (now shared) Ldweights
    # would be wrong.
    nc.move_matmul_waits_to_ldweights = lambda: None
    nc.compile()
    return nc


_NC_CACHE = None


def _get_nc():
    global _NC_CACHE
    if _NC_CACHE is None:
        _NC_CACHE = _build_nc()
    return _NC_CACHE


def _prep_host(inputs):
    """Host-side weight preprocessing -> per-core in_maps."""
    f = np.float32
    bf = ml_dtypes.bfloat16
    x = np.ascontiguousarray(inputs["x"], dtype=f)          # (16, 4, 2048)
    W1, b1 = inputs["W1"].astype(f), inputs["b1"].astype(f)
    W2, b2 = inputs["W2"].astype(f), inputs["b2"].astype(f)
    W3, b3 = inputs["W3"].astype(f), inputs["b3"].astype(f)
    W4, b4 = inputs["W4"].astype(f), inputs["b4"].astype(f)
    # [p, kt, m] = W.T[kt*128 + p, m]
    w2t = np.ascontiguousarray(
        W2.T.reshape(2, 128, D).transpose(1, 0, 2)).astype(bf)
    w4t = np.ascontiguousarray(
        W4.T.reshape(2, 128, D).transpose(1, 0, 2)).astype(bf)
    folds = np.concatenate([W1.T, (b1 + b2)[None, :]], axis=0).astype(bf)
    foldf = np.concatenate([W3.T, (b3 + 2.0 * b4)[None, :]],
                           axis=0).astype(bf)
    ones = np.ones((BSL, 1, L), dtype=f)
    in_maps = []
    for c in range(NCORES):
        xe = np.ascontiguousarray(np.concatenate(
            [x[c * BSL:(c + 1) * BSL], ones], axis=1)).astype(bf)
        in_maps.append(dict(xe=xe, w2t=w2t, w4t=w4t,
                            folds=folds, foldf=foldf))
    return in_maps


def _run(inputs, trace=False):
    nc = _get_nc()
    in_maps = _prep_host(inputs)
    res = run_bass_kernel_spmd(nc, in_maps, core_ids=list(range(NCORES)),
                               trace=trace)
    parts = [res.results[c]["out_loc"] for c in range(NCORES)]
    full = np.concatenate(parts, axis=0).astype(np.float32)  # (16, 256, 2048)
    out = np.ascontiguousarray(full.transpose(0, 2, 1))     # (16, 2048, 256)
    return out, res


def kernel(**inputs):
    out, _ = _run(inputs, trace=False)
    return out


if __name__ == "__main__":
    nc = _build_nc()
    print("build ok")
